# revision 15
# baseline (speedup 1.0000x reference)
"""ChronoFormer Trainium2 kernel.

Sharding: 8 cores = (batch b, query-half qh). Host side does indexing only:
per batch the sequence is permuted to [orig pos 2047, unmasked keys...,
masked...] and truncated to KEPT=1280 slots. Attention keys are only the
unmasked slots (masked keys contribute exp(-1e9)=0), and layer-1 outputs are
only ever read at unmasked slots + slot 0 (layer 2 reads keys at unmasked
slots and the single query at orig pos 2047), so the whole layer-1
computation runs on the KEPT range only. Core (b,1) gets the kept range
rotated by 640 so that "my queries" are always slots 0..639 — the SPMD
program is identical on all cores; only data differs. One pairwise AllGather
exchanges the layer-1 output halves, then every core (redundantly within a
pair) runs the cheap layer-2 single-query attention + classifier.

On-device layout: activations transposed (feature dim on partitions).
Scores are computed transposed (keys on partitions) so the per-key
time-bias + mask and the 1/sqrt(dk) scale fold into the scalar-engine exp
(out = exp(scale*in + bias)); softmax is unnormalized with the denominator
accumulated as a col-packed ones-row matmul, normalization applied to the
small ctx output. LayerNorm uses ones-vector matmuls for the partition-dim
sums and rsqrt = exp(-0.5*ln(var+eps)) to stay on one ACT table set.
"""

import numpy as np

B, S, D, H, DK, LAYERS = 4, 2048, 256, 4, 64, 2
V, T = 32000, 1000
KEPT = 1280
KT = KEPT // 128          # 10 key tiles
QH = KEPT // 2            # 640 queries per core
QCH = [(0, 384), (384, 256)]            # query free-dim chunks (<=512, >=256)
PCH = [(0, 512), (512, 512), (1024, 256)]  # kept-range free-dim chunks
N_CORES = 8
EPS = 1e-5
SCALE = 1.0 / np.sqrt(DK).astype(np.float32)
NEG = -1e9

_CACHE = {}


def _build():
    import concourse.bass as bass
    import concourse.mybir as mybir
    import concourse.tile as tile
    from concourse import bacc

    f32 = mybir.dt.float32
    F32R = mybir.dt.float32r
    ACT = mybir.ActivationFunctionType
    ALU = mybir.AluOpType

    nc = bacc.Bacc("TRN2", target_bir_lowering=False, debug=True,
                   num_devices=N_CORES)

    he_d = nc.dram_tensor("he", [128, 2, KEPT], f32, kind="ExternalInput")
    ht_d = nc.dram_tensor("ht", [128, 2, KEPT], f32, kind="ExternalInput")
    b1_d = nc.dram_tensor("b1", [128, H * KT], f32, kind="ExternalInput")
    b2_d = nc.dram_tensor("b2", [128, H * KT], f32, kind="ExternalInput")
    wall_d = nc.dram_tensor("wall", [128, LAYERS * 4 * 2 * D], F32R,
                            kind="ExternalInput")
    ball_d = nc.dram_tensor("ball", [128, LAYERS * 4 * 2], f32,
                            kind="ExternalInput")
    bvrow_d = nc.dram_tensor("bvrow", [1, LAYERS, D], f32, kind="ExternalInput")
    lngb_d = nc.dram_tensor("lngb", [128, LAYERS * 2 * 2], f32,
                            kind="ExternalInput")
    wc1_d = nc.dram_tensor("wc1", [128, 2 * 128], F32R, kind="ExternalInput")
    bc1_d = nc.dram_tensor("bc1", [128, 1], f32, kind="ExternalInput")
    wc2_d = nc.dram_tensor("wc2", [128, 1], F32R, kind="ExternalInput")
    bc2_d = nc.dram_tensor("bc2", [1, 1], f32, kind="ExternalInput")
    onec_d = nc.dram_tensor("onec", [128, 1], F32R, kind="ExternalInput")
    out_d = nc.dram_tensor("out", [1, 1], f32, kind="ExternalOutput")

    def r(ap):
        return ap.bitcast(F32R)

    def c(ap):
        return ap.bitcast(mybir.dt.float32)

    with tile.TileContext(nc) as tc:
        with (
            tc.tile_pool(name="const", bufs=1) as cp,
            tc.tile_pool(name="work", bufs=1) as wp,
            tc.tile_pool(name="exp", bufs=4) as ep,
            tc.tile_pool(name="tmp", bufs=2) as tp,
            tc.tile_pool(name="rows", bufs=1) as rp,
            tc.tile_pool(name="dram", bufs=1, space="DRAM") as dp,
        ):
            # ---- constant / input loads ----
            wall = cp.tile([128, LAYERS * 4 * 2 * D], F32R, tag="wall")
            nc.sync.dma_start(wall[:], wall_d[:])
            ball = cp.tile([128, LAYERS * 4 * 2], f32, tag="ball")
            nc.sync.dma_start(ball[:], ball_d[:])
            bvrow = cp.tile([1, LAYERS, D], f32, tag="bvrow")
            nc.sync.dma_start(bvrow[:], bvrow_d[:])
            lngb = cp.tile([128, LAYERS * 2 * 2], f32, tag="lngb")
            nc.sync.dma_start(lngb[:], lngb_d[:])
            b1 = cp.tile([128, H * KT], f32, tag="b1")
            nc.sync.dma_start(b1[:], b1_d[:])
            b2 = cp.tile([128, H * KT], f32, tag="b2")
            nc.sync.dma_start(b2[:], b2_d[:])
            wc1 = cp.tile([128, 2 * 128], F32R, tag="wc1")
            nc.sync.dma_start(wc1[:], wc1_d[:])
            bc1 = cp.tile([128, 1], f32, tag="bc1")
            nc.sync.dma_start(bc1[:], bc1_d[:])
            wc2 = cp.tile([128, 1], F32R, tag="wc2")
            nc.sync.dma_start(wc2[:], wc2_d[:])
            bc2 = cp.tile([1, 1], f32, tag="bc2")
            nc.sync.dma_start(bc2[:], bc2_d[:])
            ones_col = cp.tile([128, 1], F32R, tag="ones")
            nc.sync.dma_start(ones_col[:], onec_d[:])
            eps_t = cp.tile([1, 1], f32, tag="eps")
            nc.vector.memset(eps_t[:], EPS)

            he = wp.tile([128, 2, KEPT], f32, tag="he")
            nc.sync.dma_start(he[:], he_d[:])
            ht = wp.tile([128, 2, KEPT], f32, tag="ht")
            nc.sync.dma_start(ht[:], ht_d[:])

            def Wl(l, p, kc, mc):
                base = (((l * 4 + p) * 2 + kc) * D) + mc * 128
                return wall[:, base:base + 128]

            def Wfull(l, p, kc):
                base = ((l * 4 + p) * 2 + kc) * D
                return wall[:, base:base + D]

            def bl(l, p, mc):
                c = (l * 4 + p) * 2 + mc
                return ball[:, c:c + 1]

            def gb(l, g, kc):
                c = (l * 2 + g) * 2 + kc
                return lngb[:, c:c + 1]

            # embedding add
            h0 = wp.tile([128, 2, KEPT], F32R, tag="h0")
            nc.vector.tensor_tensor(out=h0[:], in0=he[:], in1=ht[:],
                                    op=ALU.add)

            def proj_T(l, p, rhs, chunks, out_sb, psum_pool, ptag):
                # transposed-output projection: out[dout, s] over given chunks
                for mc in range(2):
                    for (off, n) in chunks:
                        ps = psum_pool.tile([128, n], f32, tag=ptag)
                        for kc in range(2):
                            nc.tensor.matmul(
                                ps[:], r(Wl(l, p, kc, mc)),
                                r(rhs[:, kc, off:off + n]),
                                start=(kc == 0), stop=(kc == 1))
                        nc.vector.tensor_scalar(
                            out=out_sb[:, mc, off:off + n], in0=ps[:],
                            scalar1=bl(l, p, mc), scalar2=None, op0=ALU.add)

            def proj_V(l, rhs, out_sb, bvb, psum_pool, ptag):
                # natural-output V projection: out[s_tile, h, 0:64] with
                # out[..., 64] left as the ones column (denominator trick)
                for st in range(KT):
                    ps = psum_pool.tile([128, D], f32, tag=ptag)
                    for kc in range(2):
                        nc.tensor.matmul(
                            ps[:], r(rhs[:, kc, st * 128:(st + 1) * 128]),
                            r(Wfull(l, 2, kc)),
                            start=(kc == 0), stop=(kc == 1))
                    nc.vector.tensor_tensor(
                        out=out_sb[:, st, :, 0:64],
                        in0=ps[:].rearrange("p (h d) -> p h d", d=64),
                        in1=bvb[:].rearrange("p (h d) -> p h d", d=64),
                        op=ALU.add)

            def layer_norm_T(l, ha, sq, out_sb, ncols, chunks, stat_pool):
                # stats + apply from ha/sq [128, 2, ncols] SBUF
                m_row = rp.tile([1, ncols], f32, tag=f"m{l}")
                v_row = rp.tile([1, ncols], f32, tag=f"v{l}")
                for (off, n) in chunks:
                    ssum = stat_pool.tile([1, n], f32, tag="sts")
                    ssq = stat_pool.tile([1, n], f32, tag="stq")
                    g_ = r if n >= 256 else c
                    for kc in range(2):
                        nc.tensor.matmul(ssum[:], g_(ones_col[:]),
                                         g_(ha[:, kc, off:off + n]),
                                         start=(kc == 0), stop=(kc == 1))
                        nc.tensor.matmul(ssq[:], g_(ones_col[:]),
                                         g_(sq[:, kc, off:off + n]),
                                         start=(kc == 0), stop=(kc == 1))
                    nc.vector.tensor_scalar(out=m_row[:, off:off + n],
                                            in0=ssum[:], scalar1=1.0 / D,
                                            scalar2=None, op0=ALU.mult)
                    nc.vector.tensor_scalar(out=v_row[:, off:off + n],
                                            in0=ssq[:], scalar1=1.0 / D,
                                            scalar2=None, op0=ALU.mult)
                msq = rp.tile([1, ncols], f32, tag=f"msq{l}")
                nc.vector.tensor_tensor(out=msq[:], in0=m_row[:], in1=m_row[:],
                                        op=ALU.mult)
                nc.vector.tensor_tensor(out=v_row[:], in0=v_row[:], in1=msq[:],
                                        op=ALU.subtract)
                vln = rp.tile([1, ncols], f32, tag=f"vln{l}")
                nc.scalar.activation(vln[:], v_row[:], ACT.Ln,
                                     bias=eps_t[0:1, 0:1])
                inv_row = rp.tile([1, ncols], f32, tag=f"inv{l}")
                nc.scalar.activation(inv_row[:], vln[:], ACT.Exp, scale=-0.5)
                m2_row = rp.tile([1, ncols], f32, tag=f"m2{l}")
                nc.vector.scalar_tensor_tensor(
                    out=m2_row[:], in0=m_row[:], scalar=-1.0, in1=inv_row[:],
                    op0=ALU.mult, op1=ALU.mult)
                invb = tp.tile([128, ncols], f32, tag="invb")
                nc.gpsimd.partition_broadcast(invb[:], inv_row[:])
                m2b = tp.tile([128, ncols], f32, tag="m2b")
                nc.gpsimd.partition_broadcast(m2b[:], m2_row[:])
                for kc in range(2):
                    t1 = tp.tile([128, ncols], f32, tag="t1")
                    nc.vector.tensor_tensor(out=t1[:], in0=ha[:, kc, :],
                                            in1=invb[:], op=ALU.mult)
                    t2 = tp.tile([128, ncols], f32, tag="t2")
                    nc.vector.tensor_tensor(out=t2[:], in0=t1[:], in1=m2b[:],
                                            op=ALU.add)
                    nc.vector.tensor_scalar(
                        out=out_sb[:, kc, :], in0=t2[:],
                        scalar1=gb(l, 0, kc), scalar2=gb(l, 1, kc),
                        op0=ALU.mult, op1=ALU.add)

            # ================= LAYER 1 =================
            kT = wp.tile([128, 2, KEPT], F32R, tag="kT")
            qT = wp.tile([128, 2, QH], F32R, tag="qT")
            vN = wp.tile([128, KT, H, 65], F32R, tag="vN")
            nc.vector.tensor_copy(
                vN[:, :, :, 64:65],
                ones_col[:, 0:1].unsqueeze(1).broadcast_to([128, KT, H, 1]))
            bvb1 = wp.tile([128, D], f32, tag="bvb1")
            nc.gpsimd.partition_broadcast(bvb1[:], bvrow[0:1, 0, :])

            with tc.tile_pool(name="psP1", bufs=2, space="PSUM") as pp1:
                proj_T(0, 1, h0, PCH, kT, pp1, "pk")
                proj_T(0, 0, h0, QCH, qT, pp1, "pq")
                proj_V(0, h0, vN, bvb1, pp1, "pv")

            ctxT = wp.tile([128, 2, QH], F32R, tag="ctxT")
            with tc.tile_pool(name="psP2", bufs=1, space="PSUM") as pp2:
                for (qoff, qn) in QCH:
                    for hp in range(2):
                        mc = hp
                        ctx_ps = []
                        for hh in range(2):
                            ctx_ps.append(pp2.tile(
                                [128, qn], f32, name=f"ctx{hh}",
                                tag=f"ctx{hh}", bufs=2))
                        for kt in range(KT):
                            for hh in range(2):
                                h = hp * 2 + hh
                                hr = slice(hh * 64, hh * 64 + 64)
                                s_ps = pp2.tile([128, qn], f32, tag="s",
                                                bufs=3)
                                nc.tensor.matmul(
                                    s_ps[:],
                                    r(kT[hr, mc, kt * 128:(kt + 1) * 128]),
                                    r(qT[hr, mc, qoff:qoff + qn]),
                                    start=True, stop=True,
                                    tile_position=(hh * 64, 0))
                                e_sb = ep.tile([128, qn], F32R, tag="e")
                                nc.scalar.activation(
                                    e_sb[:], s_ps[:], ACT.Exp,
                                    bias=b1[:, h * KT + kt:h * KT + kt + 1],
                                    scale=float(SCALE))
                                nc.tensor.matmul(
                                    ctx_ps[hh][0:65, :],
                                    r(vN[:, kt, h, :]),
                                    r(e_sb[:]),
                                    start=(kt == 0), stop=(kt == KT - 1))
                        for hh in range(2):
                            r_sb = rp.tile([1, qn], f32, tag="r1", bufs=2)
                            nc.vector.reciprocal(r_sb[:],
                                                 ctx_ps[hh][64:65, :])
                            rb = tp.tile([64, qn], f32, tag="rb")
                            nc.gpsimd.partition_broadcast(rb[:], r_sb[:])
                            nc.vector.tensor_tensor(
                                out=ctxT[hh * 64:hh * 64 + 64, hp,
                                         qoff:qoff + qn],
                                in0=ctx_ps[hh][0:64, :], in1=rb[:],
                                op=ALU.mult)

            ha1 = wp.tile([128, 2, QH], F32R, tag="ha1")
            sq1 = wp.tile([128, 2, QH], F32R, tag="sq1")
            h1loc = wp.tile([128, 2, QH], F32R, tag="h1loc")
            with tc.tile_pool(name="psP3", bufs=2, space="PSUM") as pp3:
                for mc in range(2):
                    for (qoff, qn) in QCH:
                        ps = pp3.tile([128, qn], f32, tag="wo")
                        for kc in range(2):
                            nc.tensor.matmul(ps[:], r(Wl(0, 3, kc, mc)),
                                             r(ctxT[:, kc, qoff:qoff + qn]),
                                             start=(kc == 0), stop=(kc == 1))
                        nc.vector.tensor_scalar(
                            out=ha1[:, mc, qoff:qoff + qn], in0=ps[:],
                            scalar1=bl(0, 3, mc), scalar2=None, op0=ALU.add)
                        nc.scalar.activation(sq1[:, mc, qoff:qoff + qn],
                                             ps[:], ACT.Square,
                                             bias=bl(0, 3, mc))
                layer_norm_T(0, ha1, sq1, h1loc, QH, QCH, pp3)

            # ================= ALLGATHER (pairs) =================
            gin = dp.tile([2 * 128, QH], F32R, tag="gin")
            gout = dp.tile([4 * 128, QH], F32R, tag="gout")
            for kc in range(2):
                nc.sync.dma_start(gin[kc * 128:(kc + 1) * 128, :],
                                  h1loc[:, kc, :])
            nc.gpsimd.collective_compute(
                "AllGather", mybir.AluOpType.bypass,
                replica_groups=[[0, 1], [2, 3], [4, 5], [6, 7]],
                ins=[gin.opt()], outs=[gout.opt()])
            h1 = wp.tile([128, 2, KEPT], F32R, tag="h1")
            for half in range(2):
                for kc in range(2):
                    nc.sync.dma_start(
                        h1[:, kc, half * QH:(half + 1) * QH],
                        gout[half * 256 + kc * 128:half * 256 + (kc + 1) * 128, :])

            # ================= LAYER 2 =================
            k2T = wp.tile([128, 2, KEPT], F32R, tag="k2T")
            v2N = wp.tile([128, KT, H, 65], F32R, tag="v2N")
            nc.vector.tensor_copy(
                v2N[:, :, :, 64:65],
                ones_col[:, 0:1].unsqueeze(1).broadcast_to([128, KT, H, 1]))
            q2 = wp.tile([128, 2, 1], F32R, tag="q2")
            bvb2 = wp.tile([128, D], f32, tag="bvb2")
            nc.gpsimd.partition_broadcast(bvb2[:], bvrow[0:1, 1, :])
            with tc.tile_pool(name="psP4", bufs=2, space="PSUM") as pp4:
                proj_T(1, 1, h1, PCH, k2T, pp4, "pk2")
                proj_V(1, h1, v2N, bvb2, pp4, "pv2")
                for mc in range(2):
                    ps = pp4.tile([128, 1], f32, tag="pq2")
                    for kc in range(2):
                        nc.tensor.matmul(ps[:], c(Wl(1, 0, kc, mc)),
                                         c(h1[:, kc, 0:1]),
                                         start=(kc == 0), stop=(kc == 1))
                    nc.vector.tensor_scalar(out=q2[:, mc, :], in0=ps[:],
                                            scalar1=bl(1, 0, mc),
                                            scalar2=None, op0=ALU.add)

            ctx2T = wp.tile([128, 2, 1], F32R, tag="ctx2T")
            exp2 = wp.tile([128, H, KT], F32R, tag="exp2")
            with tc.tile_pool(name="psP5", bufs=2, space="PSUM") as pp5:
                for hp in range(2):
                    mc = hp
                    for hh in range(2):
                        h = hp * 2 + hh
                        hr = slice(hh * 64, hh * 64 + 64)
                        s2_ps = pp5.tile([128, KT], f32, tag="s2")
                        for kt in range(KT):
                            nc.tensor.matmul(
                                s2_ps[:, kt:kt + 1],
                                c(k2T[hr, mc, kt * 128:(kt + 1) * 128]),
                                c(q2[hr, mc, :]), start=True, stop=True,
                                tile_position=(hh * 64, 0))
                        s2e = tp.tile([128, KT], f32, tag="s2e")
                        nc.vector.scalar_tensor_tensor(
                            out=s2e[:], in0=s2_ps[:], scalar=float(SCALE),
                            in1=b2[:, h * KT:(h + 1) * KT],
                            op0=ALU.mult, op1=ALU.add)
                        nc.scalar.activation(exp2[:, h, :], s2e[:], ACT.Exp)
                        c2_ps = pp5.tile([128, 1], f32, tag="c2")
                        for kt in range(KT):
                            nc.tensor.matmul(
                                c2_ps[0:65, :],
                                c(v2N[:, kt, h, :]),
                                c(exp2[:, h, kt:kt + 1]),
                                start=(kt == 0), stop=(kt == KT - 1))
                        r2 = rp.tile([1, 1], f32, tag="r2", bufs=2)
                        nc.vector.reciprocal(r2[:], c2_ps[64:65, :])
                        r2b = tp.tile([64, 1], f32, tag="r2b")
                        nc.gpsimd.partition_broadcast(r2b[:], r2[:])
                        nc.vector.tensor_tensor(
                            out=ctx2T[hh * 64:hh * 64 + 64, hp, :],
                            in0=c2_ps[0:64, :], in1=r2b[:], op=ALU.mult)

            h2 = wp.tile([128, 2, 1], F32R, tag="h2")
            sq2 = wp.tile([128, 2, 1], F32R, tag="sq2")
            h2n = wp.tile([128, 2, 1], F32R, tag="h2n")
            with tc.tile_pool(name="psP6", bufs=1, space="PSUM") as pp6:
                for mc in range(2):
                    ps = pp6.tile([128, 1], f32, tag="wo2", bufs=2)
                    for kc in range(2):
                        nc.tensor.matmul(ps[:], c(Wl(1, 3, kc, mc)),
                                         c(ctx2T[:, kc, :]),
                                         start=(kc == 0), stop=(kc == 1))
                    nc.vector.tensor_scalar(
                        out=h2[:, mc, :], in0=ps[:],
                        scalar1=bl(1, 3, mc), scalar2=None, op0=ALU.add)
                    nc.scalar.activation(sq2[:, mc, :], ps[:], ACT.Square,
                                         bias=bl(1, 3, mc))
                layer_norm_T(1, h2, sq2, h2n, 1, [(0, 1)], pp6)

                # classifier
                hid_ps = pp6.tile([128, 1], f32, tag="hid")
                for kc in range(2):
                    nc.tensor.matmul(hid_ps[:],
                                     c(wc1[:, kc * 128:(kc + 1) * 128]),
                                     c(h2n[:, kc, :]),
                                     start=(kc == 0), stop=(kc == 1))
                hid = wp.tile([128, 1], F32R, tag="hid_sb")
                nc.scalar.activation(hid[:], hid_ps[:], ACT.Relu,
                                     bias=bc1[:, 0:1])
                z_ps = pp6.tile([1, 1], f32, tag="z")
                nc.tensor.matmul(z_ps[:], c(wc2[:]), c(hid[:]),
                                 start=True, stop=True)
                nbc2 = rp.tile([1, 1], f32, tag="nbc2")
                nc.vector.tensor_scalar(out=nbc2[:], in0=bc2[:], scalar1=-1.0,
                                        scalar2=None, op0=ALU.mult)
                ez = rp.tile([1, 1], f32, tag="ez")
                nc.scalar.activation(ez[:], z_ps[:], ACT.Exp, scale=-1.0,
                                     bias=nbc2[:])
                den = rp.tile([1, 1], f32, tag="den")
                nc.vector.tensor_scalar(out=den[:], in0=ez[:], scalar1=1.0,
                                        scalar2=None, op0=ALU.add)
                sig = rp.tile([1, 1], f32, tag="sig")
                nc.vector.reciprocal(sig[:], den[:])
                nc.sync.dma_start(out_d[:], sig[:])

    nc.compile()
    return nc


def _get_nc():
    if "nc" not in _CACHE:
        _CACHE["nc"] = _build()
    return _CACHE["nc"]


def _chunk2(a):
    """[D, N] -> [128, 2, N] splitting dim0 into 2 partition chunks."""
    n = a.shape[1]
    return np.ascontiguousarray(
        a.reshape(2, 128, n).transpose(1, 0, 2), dtype=np.float32)


def _host_prep(x, time_deltas, mask, event_emb, time_emb, Wq, bq, Wk, bk,
               Wv, bv, time_proj, Wo, bo, ln_g, ln_b, Wc1, bc1, Wc2, bc2):
    x = np.asarray(x, np.int64)
    tb = np.clip(np.asarray(time_deltas, np.int64), 0, T - 1)
    mask = np.asarray(mask, np.int64)
    event_emb = np.asarray(event_emb, np.float32)
    time_emb = np.asarray(time_emb, np.float32)
    time_proj = np.asarray(time_proj, np.float32)

    # weights (identical on every core)
    wall = np.zeros((128, LAYERS * 4 * 2 * D), np.float32)
    ball = np.zeros((128, LAYERS * 4 * 2), np.float32)
    projs = [(Wq, bq), (Wk, bk), (Wv, bv), (Wo, bo)]
    for l in range(LAYERS):
        for p, (W, b) in enumerate(projs):
            Wl = np.asarray(W[l], np.float32)  # [D, D] din x dout
            ch = Wl.reshape(2, 128, D).transpose(1, 0, 2)  # [128, kc, dout]
            base = (l * 4 + p) * 2 * D
            wall[:, base:base + 2 * D] = ch.reshape(128, 2 * D)
            bb = np.asarray(b[l], np.float32).reshape(2, 128).T  # [128, kc]
            ball[:, (l * 4 + p) * 2:(l * 4 + p) * 2 + 2] = bb
    bvrow = np.stack([np.asarray(bv[l], np.float32) for l in range(LAYERS)])
    bvrow = bvrow.reshape(1, LAYERS, D)
    lngb = np.zeros((128, LAYERS * 2 * 2), np.float32)
    for l in range(LAYERS):
        for g, arr in enumerate([ln_g[l], ln_b[l]]):
            aa = np.asarray(arr, np.float32).reshape(2, 128).T
            lngb[:, (l * 2 + g) * 2:(l * 2 + g) * 2 + 2] = aa
    wc1 = np.asarray(Wc1, np.float32).reshape(2, 128, 128).transpose(
        1, 0, 2).reshape(128, 256)
    wc1 = np.ascontiguousarray(wc1)
    bc1a = np.asarray(bc1, np.float32).reshape(128, 1)
    wc2a = np.asarray(Wc2, np.float32).reshape(128, 1)
    bc2a = np.asarray(bc2, np.float32).reshape(1, 1)

    shared = {"wall": wall, "ball": ball, "bvrow": bvrow, "lngb": lngb,
              "wc1": wc1, "bc1": bc1a, "wc2": wc2a, "bc2": bc2a,
              "onec": np.ones((128, 1), np.float32)}

    in_maps = []
    for b_i in range(B):
        m = mask[b_i]
        last = S - 1
        idx = np.arange(S)
        unm = idx[(m != 0) & (idx != last)]
        msk = idx[(m == 0) & (idx != last)]
        assert 1 + len(unm) <= KEPT, f"kept overflow: {1 + len(unm)} > {KEPT}"
        order = np.concatenate([[last], unm, msk])[:KEPT]

        e_rows = event_emb[x[b_i][order]]      # [KEPT, D]
        t_rows = time_emb[tb[b_i][order]]      # [KEPT, D]
        maskpen = np.where(m[order] == 0, np.float32(NEG), np.float32(0.0))

        def bias_dev(l):
            bias = time_proj[l][tb[b_i][order]] + maskpen[:, None]  # [KEPT,H]
            bb = bias.reshape(KT, 128, H).transpose(1, 2, 0)  # [p, h, kt]
            return np.ascontiguousarray(bb.reshape(128, H * KT), np.float32)

        b2_dev = bias_dev(1)  # layer 2 uses the global kept order
        for qh in range(2):
            if qh == 0:
                rot = np.arange(KEPT)
            else:
                rot = np.concatenate([np.arange(QH, KEPT), np.arange(QH)])
            ro = order[rot]
            he_dev = _chunk2(event_emb[x[b_i][ro]].T)
            ht_dev = _chunk2(time_emb[tb[b_i][ro]].T)
            bias1 = time_proj[0][tb[b_i][ro]] + \
                np.where(m[ro] == 0, np.float32(NEG), np.float32(0.0))[:, None]
            bb1 = bias1.reshape(KT, 128, H).transpose(1, 2, 0)
            b1_dev = np.ascontiguousarray(bb1.reshape(128, H * KT), np.float32)
            in_maps.append({"he": he_dev, "ht": ht_dev, "b1": b1_dev,
                            "b2": b2_dev, **shared})
    return in_maps


def kernel(**inputs):
    from concourse.bass_utils import run_bass_kernel_spmd
    nc = _get_nc()
    in_maps = _host_prep(**inputs)
    res = run_bass_kernel_spmd(nc, in_maps, list(range(N_CORES)))
    out = np.zeros((B, 1), np.float32)
    for b_i in range(B):
        out[b_i, 0] = res.results[2 * b_i]["out"][0, 0]
    return out


# revision 18
# speedup vs baseline: 1.5035x; 1.5035x over previous
"""ChronoFormer Trainium2 kernel.

Sharding: batch-parallel, core pairs (2b, 2b+1) redundantly compute batch b
(no collectives — a pairwise AllGather measured 94us of dead link time, more
than the compute it saved). Host side does indexing only: per batch the
sequence is permuted to [orig pos 2047, unmasked keys..., masked...] and
truncated to KEPT=1280 slots. Masked keys contribute exp(-1e9)=0 via the
per-key bias, and layer outputs are only ever read at unmasked slots +
slot 0 (layer 2 needs keys at unmasked slots and the single query at orig
pos 2047), so the whole model runs on the KEPT range.

On-device: activations transposed (feature dim on partitions), bf16 matmul
operands with fp32 PSUM accumulation. Scores are computed transposed (keys
on partitions) so the per-key time-bias + mask and the 1/sqrt(dk) scale fold
into the scalar-engine exp (out = exp(scale*in + bias)); the two 512-wide
query chunks of one (key-tile, head) share that bias, so their exp runs as
one strided ACT call over a 2-bank PSUM tile. Softmax stays unnormalized
through attn@V via a ones-column augmentation of V (row 64 of the ctx PSUM
accumulates the denominator); normalization is applied to the small ctx.
LayerNorm: ones-vector matmuls (fp32r) for partition sums, inv-std as
exp(-0.5*ln(var+eps)) — the whole kernel stays on one ACT table set
(natural_log_exp_and_others); the final sigmoid is exp + DVE reciprocal.
"""

import numpy as np

B, S, D, H, DK, LAYERS = 4, 2048, 256, 4, 64, 2
V, T = 32000, 1000
KEPT = 1280
KT = KEPT // 128          # 10 key tiles
PCH = [(0, 512), (512, 512), (1024, 256)]  # kept-range free-dim chunks
N_CORES = 8
EPS = 1e-5
SCALE = 1.0 / np.sqrt(DK).astype(np.float32)
NEG = -1e9

_CACHE = {}


def _build():
    import concourse.bass as bass
    import concourse.mybir as mybir
    import concourse.tile as tile
    from concourse import bacc

    f32 = mybir.dt.float32
    F32R = mybir.dt.float32r
    BF16 = mybir.dt.bfloat16
    ACT = mybir.ActivationFunctionType
    ALU = mybir.AluOpType

    nc = bacc.Bacc("TRN2", target_bir_lowering=False, debug=True,
                   num_devices=N_CORES)

    he_d = nc.dram_tensor("he", [128, 2, KEPT], f32, kind="ExternalInput")
    ht_d = nc.dram_tensor("ht", [128, 2, KEPT], f32, kind="ExternalInput")
    b1_d = nc.dram_tensor("b1", [128, H * KT], f32, kind="ExternalInput")
    b2_d = nc.dram_tensor("b2", [128, H * KT], f32, kind="ExternalInput")
    wall_d = nc.dram_tensor("wall", [128, LAYERS * 4 * 2 * D], BF16,
                            kind="ExternalInput")
    ball_d = nc.dram_tensor("ball", [128, LAYERS * 4 * 2], f32,
                            kind="ExternalInput")
    bvrow_d = nc.dram_tensor("bvrow", [1, LAYERS, D], f32, kind="ExternalInput")
    lngb_d = nc.dram_tensor("lngb", [128, LAYERS * 2 * 2], f32,
                            kind="ExternalInput")
    wc1_d = nc.dram_tensor("wc1", [128, 2 * 128], BF16, kind="ExternalInput")
    bc1_d = nc.dram_tensor("bc1", [128, 1], f32, kind="ExternalInput")
    wc2_d = nc.dram_tensor("wc2", [128, 1], BF16, kind="ExternalInput")
    bc2_d = nc.dram_tensor("bc2", [1, 1], f32, kind="ExternalInput")
    onec_d = nc.dram_tensor("onec", [128, 1], F32R, kind="ExternalInput")
    oneb_d = nc.dram_tensor("oneb", [128, 1], BF16, kind="ExternalInput")
    out_d = nc.dram_tensor("out", [1, 1], f32, kind="ExternalOutput")

    def r(ap):
        return ap.bitcast(F32R)

    with tile.TileContext(nc) as tc:
        with (
            tc.tile_pool(name="const", bufs=1) as cp,
            tc.tile_pool(name="work", bufs=1) as wp,
            tc.tile_pool(name="exp", bufs=4) as ep,
            tc.tile_pool(name="tmp", bufs=2) as tp,
            tc.tile_pool(name="rows", bufs=1) as rp,
        ):
            # ---- constant / input loads ----
            wall = cp.tile([128, LAYERS * 4 * 2 * D], BF16, tag="wall")
            nc.sync.dma_start(wall[:], wall_d[:])
            ball = cp.tile([128, LAYERS * 4 * 2], f32, tag="ball")
            nc.sync.dma_start(ball[:], ball_d[:])
            bvrow = cp.tile([1, LAYERS, D], f32, tag="bvrow")
            nc.sync.dma_start(bvrow[:], bvrow_d[:])
            lngb = cp.tile([128, LAYERS * 2 * 2], f32, tag="lngb")
            nc.sync.dma_start(lngb[:], lngb_d[:])
            b1 = cp.tile([128, H * KT], f32, tag="b1")
            nc.sync.dma_start(b1[:], b1_d[:])
            b2 = cp.tile([128, H * KT], f32, tag="b2")
            nc.sync.dma_start(b2[:], b2_d[:])
            wc1 = cp.tile([128, 2 * 128], BF16, tag="wc1")
            nc.sync.dma_start(wc1[:], wc1_d[:])
            bc1 = cp.tile([128, 1], f32, tag="bc1")
            nc.sync.dma_start(bc1[:], bc1_d[:])
            wc2 = cp.tile([128, 1], BF16, tag="wc2")
            nc.sync.dma_start(wc2[:], wc2_d[:])
            bc2 = cp.tile([1, 1], f32, tag="bc2")
            nc.sync.dma_start(bc2[:], bc2_d[:])
            ones_col = cp.tile([128, 1], F32R, tag="ones")
            nc.sync.dma_start(ones_col[:], onec_d[:])
            oneb = cp.tile([128, 1], BF16, tag="oneb")
            nc.sync.dma_start(oneb[:], oneb_d[:])
            eps_t = cp.tile([1, 1], f32, tag="eps")
            nc.vector.memset(eps_t[:], EPS)

            he = wp.tile([128, 2, KEPT], f32, tag="he")
            nc.sync.dma_start(he[:], he_d[:])
            ht = wp.tile([128, 2, KEPT], f32, tag="ht")
            nc.sync.dma_start(ht[:], ht_d[:])

            def Wl(l, p, kc, mc):
                base = (((l * 4 + p) * 2 + kc) * D) + mc * 128
                return wall[:, base:base + 128]

            def Wfull(l, p, kc):
                base = ((l * 4 + p) * 2 + kc) * D
                return wall[:, base:base + D]

            def bl(l, p, mc):
                c = (l * 4 + p) * 2 + mc
                return ball[:, c:c + 1]

            def gb(l, g, kc):
                c = (l * 2 + g) * 2 + kc
                return lngb[:, c:c + 1]

            # embedding add (rounds to bf16 on write)
            h0 = wp.tile([128, 2, KEPT], BF16, tag="h0")
            nc.vector.tensor_tensor(out=h0[:], in0=he[:], in1=ht[:],
                                    op=ALU.add)

            def proj_T(l, p, rhs, chunks, out_sb, psum_pool, ptag):
                # transposed-output projection: out[dout, s] over given chunks
                for mc in range(2):
                    for (off, n) in chunks:
                        ps = psum_pool.tile([128, n], f32, tag=ptag)
                        for kc in range(2):
                            nc.tensor.matmul(
                                ps[:], Wl(l, p, kc, mc),
                                rhs[:, kc, off:off + n],
                                start=(kc == 0), stop=(kc == 1))
                        nc.vector.tensor_scalar(
                            out=out_sb[:, mc, off:off + n], in0=ps[:],
                            scalar1=bl(l, p, mc), scalar2=None, op0=ALU.add)

            def proj_V(l, rhs, out_sb, bvb, psum_pool, ptag):
                # natural-output V projection into [s_tile, h, 0:64];
                # col 64 holds the ones column (softmax denominator trick)
                for st in range(KT):
                    ps = psum_pool.tile([128, D], f32, tag=ptag)
                    for kc in range(2):
                        nc.tensor.matmul(
                            ps[:], rhs[:, kc, st * 128:(st + 1) * 128],
                            Wfull(l, 2, kc),
                            start=(kc == 0), stop=(kc == 1))
                    nc.vector.tensor_tensor(
                        out=out_sb[:, st, :, 0:64],
                        in0=ps[:].rearrange("p (h d) -> p h d", d=64),
                        in1=bvb[:].rearrange("p (h d) -> p h d", d=64),
                        op=ALU.add)

            def layer_norm_T(l, ha, sq, out_sb, ncols, chunks, stat_pool):
                # stats + apply; ha/sq [128, 2, ncols] f32r SBUF
                m_row = rp.tile([1, ncols], f32, tag=f"m{l}")
                v_row = rp.tile([1, ncols], f32, tag=f"v{l}")
                for (off, n) in chunks:
                    ssum = stat_pool.tile([1, n], f32, tag="sts")
                    ssq = stat_pool.tile([1, n], f32, tag="stq")
                    g_ = r if n >= 256 else (lambda a: a.bitcast(f32))
                    for kc in range(2):
                        nc.tensor.matmul(ssum[:], g_(ones_col[:]),
                                         g_(ha[:, kc, off:off + n]),
                                         start=(kc == 0), stop=(kc == 1))
                        nc.tensor.matmul(ssq[:], g_(ones_col[:]),
                                         g_(sq[:, kc, off:off + n]),
                                         start=(kc == 0), stop=(kc == 1))
                    nc.vector.tensor_scalar(out=m_row[:, off:off + n],
                                            in0=ssum[:], scalar1=1.0 / D,
                                            scalar2=None, op0=ALU.mult)
                    nc.vector.tensor_scalar(out=v_row[:, off:off + n],
                                            in0=ssq[:], scalar1=1.0 / D,
                                            scalar2=None, op0=ALU.mult)
                msq = rp.tile([1, ncols], f32, tag=f"msq{l}")
                nc.vector.tensor_tensor(out=msq[:], in0=m_row[:], in1=m_row[:],
                                        op=ALU.mult)
                nc.vector.tensor_tensor(out=v_row[:], in0=v_row[:], in1=msq[:],
                                        op=ALU.subtract)
                vln = rp.tile([1, ncols], f32, tag=f"vln{l}")
                nc.scalar.activation(vln[:], v_row[:], ACT.Ln,
                                     bias=eps_t[0:1, 0:1])
                inv_row = rp.tile([1, ncols], f32, tag=f"inv{l}")
                nc.scalar.activation(inv_row[:], vln[:], ACT.Exp, scale=-0.5)
                m2_row = rp.tile([1, ncols], f32, tag=f"m2{l}")
                nc.vector.scalar_tensor_tensor(
                    out=m2_row[:], in0=m_row[:], scalar=-1.0, in1=inv_row[:],
                    op0=ALU.mult, op1=ALU.mult)
                invb = tp.tile([128, ncols], f32, tag="invb")
                nc.gpsimd.partition_broadcast(invb[:], inv_row[:])
                m2b = tp.tile([128, ncols], f32, tag="m2b")
                nc.gpsimd.partition_broadcast(m2b[:], m2_row[:])
                for kc in range(2):
                    t1 = tp.tile([128, ncols], f32, tag="t1")
                    nc.vector.tensor_tensor(out=t1[:], in0=ha[:, kc, :],
                                            in1=invb[:], op=ALU.mult)
                    t2 = tp.tile([128, ncols], f32, tag="t2")
                    nc.vector.tensor_tensor(out=t2[:], in0=t1[:], in1=m2b[:],
                                            op=ALU.add)
                    nc.vector.tensor_scalar(
                        out=out_sb[:, kc, :], in0=t2[:],
                        scalar1=gb(l, 0, kc), scalar2=gb(l, 1, kc),
                        op0=ALU.mult, op1=ALU.add)

            def attn_layer(qT, bias_t, kT, vN, ctxT, psum_pool):
                # scoresT -> exp -> unnormalized ctx + denominator, per head.
                # Query chunks (0,512),(512,512) share one 2-bank PSUM tile
                # and a single strided exp call; (1024,256) is separate.
                for (qoff, qn, cw) in [(0, 512, 2), (1024, 256, 1)]:
                    for hp in range(2):
                        mc = hp
                        for hh in range(2):
                            h = hp * 2 + hh
                            hr = slice(hh * 64, hh * 64 + 64)
                            ctx_ps = []
                            for j in range(cw):
                                ctx_ps.append(psum_pool.tile(
                                    [65, qn], f32, name=f"ctx{j}",
                                    tag=f"ctx{j}", bufs=2))
                            for kt in range(KT):
                                s_ps = psum_pool.tile(
                                    [128, cw * 512], f32, tag="s", bufs=2)
                                for j in range(cw):
                                    nc.tensor.matmul(
                                        s_ps[:, j * 512:j * 512 + qn],
                                        kT[hr, mc, kt * 128:(kt + 1) * 128],
                                        qT[hr, mc,
                                           qoff + j * 512:qoff + j * 512 + qn],
                                        start=True, stop=True,
                                        tile_position=(hh * 64, 0))
                                e_sb = ep.tile([128, cw, qn], BF16, tag="e")
                                e_in = s_ps[:].rearrange(
                                    "p (c q) -> p c q", c=cw)[:, :, 0:qn]
                                nc.scalar.activation(
                                    e_sb[:], e_in, ACT.Exp,
                                    bias=bias_t[:, h * KT + kt:
                                                h * KT + kt + 1],
                                    scale=float(SCALE))
                                for j in range(cw):
                                    nc.tensor.matmul(
                                        ctx_ps[j][0:65, :],
                                        vN[:, kt, h, :], e_sb[:, j, :],
                                        start=(kt == 0),
                                        stop=(kt == KT - 1))
                            for j in range(cw):
                                qo = qoff + j * 512
                                r_sb = rp.tile([1, qn], f32, tag="r1",
                                               bufs=2)
                                nc.vector.reciprocal(r_sb[:],
                                                     ctx_ps[j][64:65, :])
                                rb = tp.tile([64, qn], f32, tag="rb")
                                nc.gpsimd.partition_broadcast(rb[:], r_sb[:])
                                nc.vector.tensor_tensor(
                                    out=ctxT[hh * 64:hh * 64 + 64, hp,
                                             qo:qo + qn],
                                    in0=ctx_ps[j][0:64, :], in1=rb[:],
                                    op=ALU.mult)

            # ================= LAYER 1 =================
            kT1 = wp.tile([128, 2, KEPT], BF16, tag="kT1")
            qT1 = wp.tile([128, 2, KEPT], BF16, tag="qT1")
            vN1 = wp.tile([128, KT, H, 65], BF16, tag="vN1")
            nc.vector.tensor_copy(
                vN1[:, :, :, 64:65],
                oneb[:, 0:1].unsqueeze(1).broadcast_to([128, KT, H, 1]))
            bvb1 = wp.tile([128, D], f32, tag="bvb1")
            nc.gpsimd.partition_broadcast(bvb1[:], bvrow[0:1, 0, :])

            with tc.tile_pool(name="psP1", bufs=2, space="PSUM") as pp1:
                proj_T(0, 1, h0, PCH, kT1, pp1, "pk")
                proj_T(0, 0, h0, PCH, qT1, pp1, "pq")
                proj_V(0, h0, vN1, bvb1, pp1, "pv")

            ctxT1 = wp.tile([128, 2, KEPT], BF16, tag="ctxT1")
            with tc.tile_pool(name="psP2", bufs=1, space="PSUM") as pp2:
                attn_layer(qT1, b1, kT1, vN1, ctxT1, pp2)

            ha1 = wp.tile([128, 2, KEPT], F32R, tag="ha1")
            sq1 = wp.tile([128, 2, KEPT], F32R, tag="sq1")
            h1 = wp.tile([128, 2, KEPT], BF16, tag="h1")
            with tc.tile_pool(name="psP3", bufs=2, space="PSUM") as pp3:
                for mc in range(2):
                    for (qoff, qn) in PCH:
                        ps = pp3.tile([128, qn], f32, tag="wo")
                        for kc in range(2):
                            nc.tensor.matmul(ps[:], Wl(0, 3, kc, mc),
                                             ctxT1[:, kc, qoff:qoff + qn],
                                             start=(kc == 0), stop=(kc == 1))
                        nc.vector.tensor_scalar(
                            out=ha1[:, mc, qoff:qoff + qn], in0=ps[:],
                            scalar1=bl(0, 3, mc), scalar2=None, op0=ALU.add)
                        nc.scalar.activation(sq1[:, mc, qoff:qoff + qn],
                                             ps[:], ACT.Square,
                                             bias=bl(0, 3, mc))
                layer_norm_T(0, ha1, sq1, h1, KEPT, PCH, pp3)

            # ================= LAYER 2 =================
            k2T = wp.tile([128, 2, KEPT], BF16, tag="k2T")
            v2N = wp.tile([128, KT, H, 65], BF16, tag="v2N")
            nc.vector.tensor_copy(
                v2N[:, :, :, 64:65],
                oneb[:, 0:1].unsqueeze(1).broadcast_to([128, KT, H, 1]))
            q2 = wp.tile([128, 2, 1], BF16, tag="q2")
            bvb2 = wp.tile([128, D], f32, tag="bvb2")
            nc.gpsimd.partition_broadcast(bvb2[:], bvrow[0:1, 1, :])
            with tc.tile_pool(name="psP4", bufs=2, space="PSUM") as pp4:
                proj_T(1, 1, h1, PCH, k2T, pp4, "pk2")
                proj_V(1, h1, v2N, bvb2, pp4, "pv2")
                for mc in range(2):
                    ps = pp4.tile([128, 1], f32, tag="pq2")
                    for kc in range(2):
                        nc.tensor.matmul(ps[:], Wl(1, 0, kc, mc),
                                         h1[:, kc, 0:1],
                                         start=(kc == 0), stop=(kc == 1))
                    nc.vector.tensor_scalar(out=q2[:, mc, :], in0=ps[:],
                                            scalar1=bl(1, 0, mc),
                                            scalar2=None, op0=ALU.add)

            ctx2T = wp.tile([128, 2, 1], BF16, tag="ctx2T")
            exp2 = wp.tile([128, H, KT], BF16, tag="exp2")
            with tc.tile_pool(name="psP5", bufs=2, space="PSUM") as pp5:
                for hp in range(2):
                    mc = hp
                    for hh in range(2):
                        h = hp * 2 + hh
                        hr = slice(hh * 64, hh * 64 + 64)
                        s2_ps = pp5.tile([128, KT], f32, tag="s2")
                        for kt in range(KT):
                            nc.tensor.matmul(
                                s2_ps[:, kt:kt + 1],
                                k2T[hr, mc, kt * 128:(kt + 1) * 128],
                                q2[hr, mc, :], start=True, stop=True,
                                tile_position=(hh * 64, 0))
                        s2e = tp.tile([128, KT], f32, tag="s2e")
                        nc.vector.scalar_tensor_tensor(
                            out=s2e[:], in0=s2_ps[:], scalar=float(SCALE),
                            in1=b2[:, h * KT:(h + 1) * KT],
                            op0=ALU.mult, op1=ALU.add)
                        nc.scalar.activation(exp2[:, h, :], s2e[:], ACT.Exp)
                        c2_ps = pp5.tile([128, 1], f32, tag="c2")
                        for kt in range(KT):
                            nc.tensor.matmul(
                                c2_ps[0:65, :],
                                v2N[:, kt, h, :],
                                exp2[:, h, kt:kt + 1],
                                start=(kt == 0), stop=(kt == KT - 1))
                        r2 = rp.tile([1, 1], f32, tag="r2", bufs=2)
                        nc.vector.reciprocal(r2[:], c2_ps[64:65, :])
                        r2b = tp.tile([64, 1], f32, tag="r2b")
                        nc.gpsimd.partition_broadcast(r2b[:], r2[:])
                        nc.vector.tensor_tensor(
                            out=ctx2T[hh * 64:hh * 64 + 64, hp, :],
                            in0=c2_ps[0:64, :], in1=r2b[:], op=ALU.mult)

            h2 = wp.tile([128, 2, 1], F32R, tag="h2")
            sq2 = wp.tile([128, 2, 1], F32R, tag="sq2")
            h2n = wp.tile([128, 2, 1], BF16, tag="h2n")
            with tc.tile_pool(name="psP6", bufs=1, space="PSUM") as pp6:
                for mc in range(2):
                    ps = pp6.tile([128, 1], f32, tag="wo2", bufs=2)
                    for kc in range(2):
                        nc.tensor.matmul(ps[:], Wl(1, 3, kc, mc),
                                         ctx2T[:, kc, :],
                                         start=(kc == 0), stop=(kc == 1))
                    nc.vector.tensor_scalar(
                        out=h2[:, mc, :], in0=ps[:],
                        scalar1=bl(1, 3, mc), scalar2=None, op0=ALU.add)
                    nc.scalar.activation(sq2[:, mc, :], ps[:], ACT.Square,
                                         bias=bl(1, 3, mc))
                layer_norm_T(1, h2, sq2, h2n, 1, [(0, 1)], pp6)

                # classifier
                hid_ps = pp6.tile([128, 1], f32, tag="hid")
                for kc in range(2):
                    nc.tensor.matmul(hid_ps[:],
                                     wc1[:, kc * 128:(kc + 1) * 128],
                                     h2n[:, kc, :],
                                     start=(kc == 0), stop=(kc == 1))
                hid = wp.tile([128, 1], BF16, tag="hid_sb")
                nc.scalar.activation(hid[:], hid_ps[:], ACT.Relu,
                                     bias=bc1[:, 0:1])
                z_ps = pp6.tile([1, 1], f32, tag="z")
                nc.tensor.matmul(z_ps[:], wc2[:], hid[:],
                                 start=True, stop=True)
                nbc2 = rp.tile([1, 1], f32, tag="nbc2")
                nc.vector.tensor_scalar(out=nbc2[:], in0=bc2[:], scalar1=-1.0,
                                        scalar2=None, op0=ALU.mult)
                ez = rp.tile([1, 1], f32, tag="ez")
                nc.scalar.activation(ez[:], z_ps[:], ACT.Exp, scale=-1.0,
                                     bias=nbc2[:])
                den = rp.tile([1, 1], f32, tag="den")
                nc.vector.tensor_scalar(out=den[:], in0=ez[:], scalar1=1.0,
                                        scalar2=None, op0=ALU.add)
                sig = rp.tile([1, 1], f32, tag="sig")
                nc.vector.reciprocal(sig[:], den[:])
                nc.sync.dma_start(out_d[:], sig[:])

    nc.compile()
    return nc


def _get_nc():
    if "nc" not in _CACHE:
        _CACHE["nc"] = _build()
    return _CACHE["nc"]


def _chunk2(a):
    """[D, N] -> [128, 2, N] splitting dim0 into 2 partition chunks."""
    n = a.shape[1]
    return np.ascontiguousarray(
        a.reshape(2, 128, n).transpose(1, 0, 2), dtype=np.float32)


def _host_prep(x, time_deltas, mask, event_emb, time_emb, Wq, bq, Wk, bk,
               Wv, bv, time_proj, Wo, bo, ln_g, ln_b, Wc1, bc1, Wc2, bc2):
    import ml_dtypes
    bf16 = ml_dtypes.bfloat16
    x = np.asarray(x, np.int64)
    tb = np.clip(np.asarray(time_deltas, np.int64), 0, T - 1)
    mask = np.asarray(mask, np.int64)
    event_emb = np.asarray(event_emb, np.float32)
    time_emb = np.asarray(time_emb, np.float32)
    time_proj = np.asarray(time_proj, np.float32)

    # weights (identical on every core)
    wall = np.zeros((128, LAYERS * 4 * 2 * D), np.float32)
    ball = np.zeros((128, LAYERS * 4 * 2), np.float32)
    projs = [(Wq, bq), (Wk, bk), (Wv, bv), (Wo, bo)]
    for l in range(LAYERS):
        for p, (W, b) in enumerate(projs):
            Wmat = np.asarray(W[l], np.float32)  # [D, D] din x dout
            ch = Wmat.reshape(2, 128, D).transpose(1, 0, 2)  # [128, kc, dout]
            base = (l * 4 + p) * 2 * D
            wall[:, base:base + 2 * D] = ch.reshape(128, 2 * D)
            bb = np.asarray(b[l], np.float32).reshape(2, 128).T  # [128, kc]
            ball[:, (l * 4 + p) * 2:(l * 4 + p) * 2 + 2] = bb
    bvrow = np.stack([np.asarray(bv[l], np.float32) for l in range(LAYERS)])
    bvrow = bvrow.reshape(1, LAYERS, D)
    lngb = np.zeros((128, LAYERS * 2 * 2), np.float32)
    for l in range(LAYERS):
        for g, arr in enumerate([ln_g[l], ln_b[l]]):
            aa = np.asarray(arr, np.float32).reshape(2, 128).T
            lngb[:, (l * 2 + g) * 2:(l * 2 + g) * 2 + 2] = aa
    wc1 = np.asarray(Wc1, np.float32).reshape(2, 128, 128).transpose(
        1, 0, 2).reshape(128, 256)
    wc1 = np.ascontiguousarray(wc1)
    bc1a = np.asarray(bc1, np.float32).reshape(128, 1)
    wc2a = np.asarray(Wc2, np.float32).reshape(128, 1)
    bc2a = np.asarray(bc2, np.float32).reshape(1, 1)

    shared = {"wall": wall.astype(bf16), "ball": ball, "bvrow": bvrow,
              "lngb": lngb, "wc1": wc1.astype(bf16), "bc1": bc1a,
              "wc2": wc2a.astype(bf16), "bc2": bc2a,
              "onec": np.ones((128, 1), np.float32),
              "oneb": np.ones((128, 1), bf16)}

    in_maps = []
    for b_i in range(B):
        m = mask[b_i]
        last = S - 1
        idx = np.arange(S)
        unm = idx[(m != 0) & (idx != last)]
        assert 1 + len(unm) <= KEPT, f"kept overflow: {1 + len(unm)} > {KEPT}"
        order = np.concatenate(
            [[last], unm, idx[(m == 0) & (idx != last)]])[:KEPT]

        he_dev = _chunk2(event_emb[x[b_i][order]].T)
        ht_dev = _chunk2(time_emb[tb[b_i][order]].T)
        maskpen = np.where(m[order] == 0, np.float32(NEG), np.float32(0.0))

        def bias_dev(l):
            bias = time_proj[l][tb[b_i][order]] + maskpen[:, None]  # [KEPT,H]
            bb = bias.reshape(KT, 128, H).transpose(1, 2, 0)  # [p, h, kt]
            return np.ascontiguousarray(bb.reshape(128, H * KT), np.float32)

        core_map = {"he": he_dev, "ht": ht_dev, "b1": bias_dev(0),
                    "b2": bias_dev(1), **shared}
        in_maps.append(core_map)
        in_maps.append(core_map)
    return in_maps


def kernel(**inputs):
    from concourse.bass_utils import run_bass_kernel_spmd
    nc = _get_nc()
    in_maps = _host_prep(**inputs)
    res = run_bass_kernel_spmd(nc, in_maps, list(range(N_CORES)))
    out = np.zeros((B, 1), np.float32)
    for b_i in range(B):
        out[b_i, 0] = res.results[2 * b_i]["out"][0, 0]
    return out
